# revision 11
# baseline (speedup 1.0000x reference)
"""Trainium2 Bass kernel for a 2-layer LIF spiking net (snntorch-style).

Reference computation (per time step t, for 100 steps):
    cur1 = x_t @ W1.T                  [B, 512]
    mem1 = 0.9*mem1 + cur1 - (mem1_prev > 1)   ; spk1 = mem1 > 1
    cur2 = spk1 @ W2.T                 [B, 10]
    mem2 = 0.8*mem2 + cur2 - (mem2_prev > 1)   ; spk2 = mem2 > 1
    record cur1, cur2, spk1, spk2, mem1, mem2

Sharding: data-parallel over batch (1024 / 8 cores = 128 = partition width).
Weights replicated. Each core runs the full 100-step recurrence on its
batch shard with membrane state resident in SBUF.

Key design points (all verified against the TRN2 cost model):
- x is split on the host into 2 fp16 terms (hi + lo = x to ~2^-22
  relative; fp16 range is ample for N(0,1) data): W1 is exactly {0, 0.5}
  in fp16, so cur1 via 4 fp16 matmuls is fp32-accurate but runs at
  1 cycle/row on the PE instead of fp32's 4.
- reset_t == spk_{t-1} (threshold 1.0, reset-by-subtract), so the previous
  spike tile doubles as the reset mask: one extra -I @ spk_bf16 matmul
  accumulates the reset into PSUM for free.
- The membrane update is ONE fused DVE op: mem = (mem_prev * beta) + psum.
- DMA issue cost is the main scalability limit: loads+cur1 on SyncE HWDGE;
  spk1+mem1 on GpSimd SWDGE (keeps ScalarE's strict FIFO free for the
  PSUM-drain on the recurrence critical path); tiny layer-2 outputs are
  staged in SBUF and flushed once per SGRP steps via SWDGE.
"""

import numpy as np
import ml_dtypes

T, B, NI, NH, NO = 100, 1024, 256, 512, 10
NCORES = 8
BS = B // NCORES  # 128 batch rows per core == SBUF partition count
BETA1, BETA2, THRESH = 0.9, 0.8, 1.0
SGRP = 20  # layer-2 output staging group (steps per DMA)

_CACHE = None


def _build_program(reps=None):
    """Build the Bass program. reps>1 wraps the computation in an on-device
    For_i loop — used only for wall-clock HW timing (amortizes the multi-ms
    PJRT/axon dispatch overhead over many executions)."""
    from contextlib import ExitStack, nullcontext

    import concourse.bass as bass
    import concourse.tile as tile
    from concourse import bacc, mybir

    f32 = mybir.dt.float32
    bf16 = mybir.dt.bfloat16

    nc = bacc.Bacc(
        "TRN2",
        target_bir_lowering=False,
        debug=False,
        enable_asserts=False,
        num_devices=NCORES,
    )

    # ---- DRAM I/O (per-core shard shapes) ----
    # x: [t][k-chunk][p][term,b] with (term,b) merged so each (t,k,p) row is
    # one contiguous 512B run: term-major blocks of 128 fp16 batch values.
    f16 = mybir.dt.float16
    x_d = nc.dram_tensor("x", [T, 2, 128, 2 * BS], f16, kind="ExternalInput").ap()
    w1t_d = nc.dram_tensor("w1t", [2, 128, NH], f16, kind="ExternalInput").ap()
    w2t_d = nc.dram_tensor("w2t", [4, 128, NO], bf16, kind="ExternalInput").ap()
    identb_d = nc.dram_tensor("identb", [128, 128], bf16, kind="ExternalInput").ap()
    inegb_d = nc.dram_tensor("inegb", [128, 128], bf16, kind="ExternalInput").ap()

    cur1_d = nc.dram_tensor("cur1", [T, BS, NH], f32, kind="ExternalOutput").ap()
    spk1_d = nc.dram_tensor("spk1", [T, BS, NH], f32, kind="ExternalOutput").ap()
    mem1_d = nc.dram_tensor("mem1", [T, BS, NH], f32, kind="ExternalOutput").ap()
    cur2_d = nc.dram_tensor("cur2", [T, BS, NO], f32, kind="ExternalOutput").ap()
    spk2_d = nc.dram_tensor("spk2", [T, BS, NO], f32, kind="ExternalOutput").ap()
    mem2_d = nc.dram_tensor("mem2", [T, BS, NO], f32, kind="ExternalOutput").ap()

    with tile.TileContext(nc) as tc:
        with ExitStack() as ctx:
            consts = ctx.enter_context(tc.tile_pool(name="consts", bufs=1))
            x_pool = ctx.enter_context(tc.tile_pool(name="x", bufs=6))
            m1_pool = ctx.enter_context(tc.tile_pool(name="m1", bufs=4))
            s1b_pool = ctx.enter_context(tc.tile_pool(name="s1b", bufs=4))
            s1f_pool = ctx.enter_context(tc.tile_pool(name="s1f", bufs=4))
            c1_pool = ctx.enter_context(tc.tile_pool(name="c1", bufs=4))
            s1t_pool = ctx.enter_context(tc.tile_pool(name="s1t", bufs=3))
            l2_pool = ctx.enter_context(tc.tile_pool(name="l2", bufs=3))
            ps1_pool = ctx.enter_context(tc.tile_pool(name="ps1", bufs=2, space="PSUM"))
            psT_pool = ctx.enter_context(tc.tile_pool(name="psT", bufs=2, space="PSUM"))
            ps2_pool = ctx.enter_context(tc.tile_pool(name="ps2", bufs=2, space="PSUM"))

            # ---- constants ----
            f16 = mybir.dt.float16
            w1t_sb = consts.tile([128, 2 * NH], f16)
            for k in range(2):
                nc.sync.dma_start(w1t_sb[:, k * NH : (k + 1) * NH], w1t_d[k])
            w2t_sb = consts.tile([128, 4 * NO], bf16)
            for j in range(4):
                nc.sync.dma_start(w2t_sb[:, j * NO : (j + 1) * NO], w2t_d[j])
            identb_sb = consts.tile([128, 128], bf16)
            nc.sync.dma_start(identb_sb[:], identb_d[:])
            inegb_sb = consts.tile([128, 128], bf16)
            nc.sync.dma_start(inegb_sb[:], inegb_d[:])

            loop_cm = tc.For_i(0, reps, 1) if reps and reps > 1 else nullcontext()
            with loop_cm:
                _emit_body(
                    nc, tc, mybir,
                    (x_pool, m1_pool, s1b_pool, s1f_pool, c1_pool, s1t_pool,
                     l2_pool, ps1_pool, psT_pool, ps2_pool),
                    (w1t_sb, w2t_sb, identb_sb, inegb_sb),
                    (x_d, cur1_d, spk1_d, mem1_d, cur2_d, spk2_d, mem2_d),
                )

    nc.compile()
    return nc


def _emit_body(nc, tc, mybir, pools, consts_sb, drams):
    f32 = mybir.dt.float32
    bf16 = mybir.dt.bfloat16
    Alu = mybir.AluOpType
    (x_pool, m1_pool, s1b_pool, s1f_pool, c1_pool, s1t_pool,
     l2_pool, ps1_pool, psT_pool, ps2_pool) = pools
    w1t_sb, w2t_sb, identb_sb, inegb_sb = consts_sb
    x_d, cur1_d, spk1_d, mem1_d, cur2_d, spk2_d, mem2_d = drams

    # ---- initial state (zeros) ----
    mem1_prev = m1_pool.tile([128, NH], f32, tag="m1")
    nc.gpsimd.memset(mem1_prev[:], 0.0)
    spk1b_prev = s1b_pool.tile([128, NH], bf16, tag="s1b")
    nc.gpsimd.memset(spk1b_prev[:], 0.0)
    # layer-2 staging tiles: SGRP steps wide, written in-place per step,
    # flushed to DRAM once per group via SWDGE
    cur2_st = l2_pool.tile([128, SGRP * NO], f32, tag="c2")
    mem2_st = l2_pool.tile([128, SGRP * NO], f32, tag="m2")
    spk2_st = l2_pool.tile([128, SGRP * NO], f32, tag="s2")
    spk2b_st = l2_pool.tile([128, SGRP * NO], bf16, tag="s2b")
    nc.gpsimd.memset(mem2_st[:], 0.0)
    nc.gpsimd.memset(spk2b_st[:], 0.0)
    mem2_prev = mem2_st[:, (SGRP - 1) * NO : SGRP * NO]
    spk2b_prev = spk2b_st[:, (SGRP - 1) * NO : SGRP * NO]

    for t in range(T):
        s = t % SGRP
        if s == 0:
            if t > 0:
                cur2_st = l2_pool.tile([128, SGRP * NO], f32, tag="c2")
                mem2_st = l2_pool.tile([128, SGRP * NO], f32, tag="m2")
                spk2_st = l2_pool.tile([128, SGRP * NO], f32, tag="s2")
                spk2b_st = l2_pool.tile([128, SGRP * NO], bf16, tag="s2b")

        # ---- layer 1: cur1 = x_t @ W1.T via 2-term fp16 split ----
        f16 = mybir.dt.float16
        xt = x_pool.tile([128, 2 * 2 * 128], f16, tag="xt")
        nc.sync.dma_start(
            xt[:].rearrange("p (k tb) -> p k tb", k=2),
            x_d[t].rearrange("k p tb -> p k tb"),
        )

        ps1 = ps1_pool.tile([128, NH], f32, tag="ps1")
        for i in range(4):  # (k, term) pairs
            k = i // 2
            nc.tensor.matmul(
                ps1[:],
                lhsT=xt[:, i * 128 : (i + 1) * 128],
                rhs=w1t_sb[:, k * NH : (k + 1) * NH],
                start=(i == 0),
                stop=(i == 3),
            )
        cur1_sb = c1_pool.tile([128, NH], f32, tag="c1")
        nc.scalar.copy(cur1_sb[:], ps1[:])
        nc.sync.dma_start(cur1_d[t], cur1_sb[:])

        # ps1 += -spk1_prev (reset-by-subtract, thresh=1). The accumulation
        # group was closed so cur1 could be read out; has_written bits are
        # still set, so start=False keeps accumulating on HW.
        nc.tensor.matmul(
            ps1[:], lhsT=inegb_sb[:], rhs=spk1b_prev[:],
            start=False, stop=True, skip_group_check=True,
        )

        # mem1 = mem1_prev*beta1 + (cur1 - spk1_prev)   [one fused DVE op]
        mem1 = m1_pool.tile([128, NH], f32, tag="m1")
        nc.vector.scalar_tensor_tensor(
            mem1[:], mem1_prev[:], BETA1, ps1[:], op0=Alu.mult, op1=Alu.add
        )
        nc.gpsimd.dma_start(mem1_d[t], mem1[:])

        # spk1 = mem1 > 1 (bf16 master; f32 cast for DRAM)
        spk1b = s1b_pool.tile([128, NH], bf16, tag="s1b")
        nc.vector.tensor_scalar(spk1b[:], mem1[:], THRESH, None, op0=Alu.is_gt)
        spk1f = s1f_pool.tile([128, NH], f32, tag="s1f")
        nc.vector.tensor_copy(spk1f[:], spk1b[:])
        nc.gpsimd.dma_start(spk1_d[t], spk1f[:])

        # ---- layer 2: cur2 = spk1 @ W2.T ----
        psT = psT_pool.tile([128, 4 * 128], bf16, tag="psT")
        for j in range(4):
            nc.tensor.matmul(
                psT[:, j * 128 : (j + 1) * 128],
                lhsT=spk1b[:, j * 128 : (j + 1) * 128],
                rhs=identb_sb[:],
                is_transpose=True, start=True, stop=True,
            )
        spk1T = s1t_pool.tile([128, 4 * 128], bf16, tag="s1t")
        nc.scalar.copy(spk1T[:], psT[:])

        ps2 = ps2_pool.tile([128, NO], f32, tag="ps2")
        for j in range(4):
            nc.tensor.matmul(
                ps2[:],
                lhsT=spk1T[:, j * 128 : (j + 1) * 128],
                rhs=w2t_sb[:, j * NO : (j + 1) * NO],
                start=(j == 0),
                stop=(j == 3),
            )
        nc.scalar.copy(cur2_st[:, s * NO : (s + 1) * NO], ps2[:])

        nc.tensor.matmul(
            ps2[:], lhsT=inegb_sb[:], rhs=spk2b_prev,
            start=False, stop=True, skip_group_check=True,
        )

        mem2_cur = mem2_st[:, s * NO : (s + 1) * NO]
        nc.vector.scalar_tensor_tensor(
            mem2_cur, mem2_prev, BETA2, ps2[:], op0=Alu.mult, op1=Alu.add
        )
        spk2_cur = spk2_st[:, s * NO : (s + 1) * NO]
        nc.vector.tensor_scalar(spk2_cur, mem2_cur, THRESH, None, op0=Alu.is_gt)
        spk2b_cur = spk2b_st[:, s * NO : (s + 1) * NO]
        nc.vector.tensor_scalar(spk2b_cur, mem2_cur, THRESH, None, op0=Alu.is_gt)

        if s == SGRP - 1:
            t0 = t - SGRP + 1
            for st_tile, dram in ((cur2_st, cur2_d), (mem2_st, mem2_d), (spk2_st, spk2_d)):
                nc.gpsimd.dma_start(
                    dram[t0 : t0 + SGRP].rearrange("t b o -> b t o"),
                    st_tile[:].rearrange("p (t o) -> p t o", t=SGRP),
                )

        mem1_prev, spk1b_prev = mem1, spk1b
        mem2_prev, spk2b_prev = mem2_cur, spk2b_cur


def _shard_inputs(x, W1, W2):
    """Host-side prep: transpose x to [T, NI, B], 2-term fp16 split, shard."""
    bf = ml_dtypes.bfloat16
    xT = np.ascontiguousarray(x.transpose(0, 2, 1))  # [T, NI, B]
    hi = xT.astype(np.float16)
    lo = (xT - hi.astype(np.float32)).astype(np.float16)
    # [T, NI, 2, B] -> [T, 2, 128, 2*B]
    xs = np.stack([hi, lo], axis=2).reshape(T, 2, 128, 2 * B)
    w1t = np.ascontiguousarray(W1.T).astype(np.float16).reshape(2, 128, NH)
    w2t = np.ascontiguousarray(W2.T).astype(bf).reshape(4, 128, NO)
    identb = np.eye(128, dtype=bf)
    inegb = (-np.eye(128)).astype(bf)
    in_maps = []
    for c in range(NCORES):
        # per-core: pick batch columns c*BS..(c+1)*BS from each term block
        xc = xs.reshape(T, 2, 128, 2, B)[:, :, :, :, c * BS : (c + 1) * BS]
        in_maps.append(
            {
                "x": np.ascontiguousarray(xc).reshape(T, 2, 128, 2 * BS),
                "w1t": w1t,
                "w2t": w2t,
                "identb": identb,
                "inegb": inegb,
            }
        )
    return in_maps


def get_program():
    global _CACHE
    if _CACHE is None:
        _CACHE = _build_program()
    return _CACHE


def kernel(x, W1, W2):
    from concourse.bass_utils import run_bass_kernel_spmd

    nc = get_program()
    in_maps = _shard_inputs(np.asarray(x), np.asarray(W1), np.asarray(W2))
    res = run_bass_kernel_spmd(nc, in_maps, core_ids=list(range(NCORES)))
    outs = res.results

    def gather(name):
        return np.concatenate([outs[c][name] for c in range(NCORES)], axis=1)

    return (
        gather("cur1"),
        gather("cur2"),
        gather("spk1"),
        gather("spk2"),
        gather("mem1"),
        gather("mem2"),
    )


# revision 12
# speedup vs baseline: 1.4237x; 1.4237x over previous
"""Trainium2 Bass kernel for a 2-layer LIF spiking net (snntorch-style).

Reference computation (per time step t, for 100 steps):
    cur1 = x_t @ W1.T                  [B, 512]
    mem1 = 0.9*mem1 + cur1 - (mem1_prev > 1)   ; spk1 = mem1 > 1
    cur2 = spk1 @ W2.T                 [B, 10]
    mem2 = 0.8*mem2 + cur2 - (mem2_prev > 1)   ; spk2 = mem2 > 1
    record cur1, cur2, spk1, spk2, mem1, mem2

Sharding: data-parallel over batch (1024 / 8 cores = 128 = partition width).
Weights replicated. Each core runs the full 100-step recurrence on its
batch shard with membrane state resident in SBUF.

Key design points (all verified against the TRN2 cost model):
- x is split on the host into 2 fp16 terms (hi + lo = x to ~2^-22
  relative; fp16 range is ample for N(0,1) data): W1 is exactly {0, 0.5}
  in fp16, so cur1 via 4 fp16 matmuls is fp32-accurate but runs at
  1 cycle/row on the PE instead of fp32's 4.
- reset_t == spk_{t-1} (threshold 1.0, reset-by-subtract), so the previous
  spike tile doubles as the reset mask: one extra -I @ spk_bf16 matmul
  accumulates the reset into PSUM for free.
- The membrane update is ONE fused DVE op: mem = (mem_prev * beta) + psum.
- DMA issue cost is the main scalability limit: loads+cur1 on SyncE HWDGE;
  spk1+mem1 on GpSimd SWDGE (keeps ScalarE's strict FIFO free for the
  PSUM-drain on the recurrence critical path); tiny layer-2 outputs are
  staged in SBUF and flushed once per SGRP steps via SWDGE.
"""

import numpy as np
import ml_dtypes

T, B, NI, NH, NO = 100, 1024, 256, 512, 10
NCORES = 8
BS = B // NCORES  # 128 batch rows per core == SBUF partition count
BETA1, BETA2, THRESH = 0.9, 0.8, 1.0
SGRP = 20  # layer-2 output staging group (steps per DMA)

_CACHE = None


def _build_program(reps=None):
    """Build the Bass program. reps>1 wraps the computation in an on-device
    For_i loop — used only for wall-clock HW timing (amortizes the multi-ms
    PJRT/axon dispatch overhead over many executions)."""
    from contextlib import ExitStack, nullcontext

    import concourse.bass as bass
    import concourse.tile as tile
    from concourse import bacc, mybir

    f32 = mybir.dt.float32
    bf16 = mybir.dt.bfloat16

    nc = bacc.Bacc(
        "TRN2",
        target_bir_lowering=False,
        debug=False,
        enable_asserts=False,
        num_devices=NCORES,
    )

    # ---- DRAM I/O (per-core shard shapes) ----
    # x: [t][k-chunk][p][term,b] with (term,b) merged so each (t,k,p) row is
    # one contiguous 512B run: term-major blocks of 128 fp16 batch values.
    f16 = mybir.dt.float16
    x_d = nc.dram_tensor("x", [T, 2, 128, 2 * BS], f16, kind="ExternalInput").ap()
    w1t_d = nc.dram_tensor("w1t", [2, 128, NH], f16, kind="ExternalInput").ap()
    w2t_d = nc.dram_tensor("w2t", [4, 128, NO], bf16, kind="ExternalInput").ap()
    identb_d = nc.dram_tensor("identb", [128, 128], bf16, kind="ExternalInput").ap()
    inegb_d = nc.dram_tensor("inegb", [128, 128], bf16, kind="ExternalInput").ap()

    cur1_d = nc.dram_tensor("cur1", [T, BS, NH], f32, kind="ExternalOutput").ap()
    spk1_d = nc.dram_tensor("spk1", [T, BS, NH], f32, kind="ExternalOutput").ap()
    mem1_d = nc.dram_tensor("mem1", [T, BS, NH], f32, kind="ExternalOutput").ap()
    cur2_d = nc.dram_tensor("cur2", [T, BS, NO], f32, kind="ExternalOutput").ap()
    spk2_d = nc.dram_tensor("spk2", [T, BS, NO], f32, kind="ExternalOutput").ap()
    mem2_d = nc.dram_tensor("mem2", [T, BS, NO], f32, kind="ExternalOutput").ap()

    with tile.TileContext(nc) as tc:
        with ExitStack() as ctx:
            consts = ctx.enter_context(tc.tile_pool(name="consts", bufs=1))
            x_pool = ctx.enter_context(tc.tile_pool(name="x", bufs=6))
            m1_pool = ctx.enter_context(tc.tile_pool(name="m1", bufs=4))
            s1b_pool = ctx.enter_context(tc.tile_pool(name="s1b", bufs=4))
            s1f_pool = ctx.enter_context(tc.tile_pool(name="s1f", bufs=4))
            c1_pool = ctx.enter_context(tc.tile_pool(name="c1", bufs=4))
            s1t_pool = ctx.enter_context(tc.tile_pool(name="s1t", bufs=3))
            l2_pool = ctx.enter_context(tc.tile_pool(name="l2", bufs=3))
            ps1_pool = ctx.enter_context(tc.tile_pool(name="ps1", bufs=2, space="PSUM"))
            psT_pool = ctx.enter_context(tc.tile_pool(name="psT", bufs=2, space="PSUM"))
            ps2_pool = ctx.enter_context(tc.tile_pool(name="ps2", bufs=2, space="PSUM"))

            # ---- constants ----
            f16 = mybir.dt.float16
            w1t_sb = consts.tile([128, 2 * NH], f16)
            for k in range(2):
                nc.sync.dma_start(w1t_sb[:, k * NH : (k + 1) * NH], w1t_d[k])
            w2t_sb = consts.tile([128, 4 * NO], bf16)
            for j in range(4):
                nc.sync.dma_start(w2t_sb[:, j * NO : (j + 1) * NO], w2t_d[j])
            identb_sb = consts.tile([128, 128], bf16)
            nc.sync.dma_start(identb_sb[:], identb_d[:])
            inegb_sb = consts.tile([128, 128], bf16)
            nc.sync.dma_start(inegb_sb[:], inegb_d[:])

            loop_cm = tc.For_i(0, reps, 1) if reps and reps > 1 else nullcontext()
            with loop_cm:
                _emit_body(
                    nc, tc, mybir,
                    (x_pool, m1_pool, s1b_pool, s1f_pool, c1_pool, s1t_pool,
                     l2_pool, ps1_pool, psT_pool, ps2_pool),
                    (w1t_sb, w2t_sb, identb_sb, inegb_sb),
                    (x_d, cur1_d, spk1_d, mem1_d, cur2_d, spk2_d, mem2_d),
                )

    nc.compile()
    return nc


def _emit_body(nc, tc, mybir, pools, consts_sb, drams):
    f32 = mybir.dt.float32
    bf16 = mybir.dt.bfloat16
    Alu = mybir.AluOpType
    (x_pool, m1_pool, s1b_pool, s1f_pool, c1_pool, s1t_pool,
     l2_pool, ps1_pool, psT_pool, ps2_pool) = pools
    w1t_sb, w2t_sb, identb_sb, inegb_sb = consts_sb
    x_d, cur1_d, spk1_d, mem1_d, cur2_d, spk2_d, mem2_d = drams

    # ---- initial state (zeros) ----
    mem1_prev = m1_pool.tile([128, NH], f32, tag="m1")
    nc.gpsimd.memset(mem1_prev[:], 0.0)
    spk1b_prev = s1b_pool.tile([128, NH], bf16, tag="s1b")
    nc.gpsimd.memset(spk1b_prev[:], 0.0)
    # layer-2 staging tiles: SGRP steps wide, written in-place per step,
    # flushed to DRAM once per group via SWDGE
    cur2_st = l2_pool.tile([128, SGRP * NO], f32, tag="c2")
    mem2_st = l2_pool.tile([128, SGRP * NO], f32, tag="m2")
    spk2_st = l2_pool.tile([128, SGRP * NO], f32, tag="s2")
    spk2b_st = l2_pool.tile([128, SGRP * NO], bf16, tag="s2b")
    nc.gpsimd.memset(mem2_st[:], 0.0)
    nc.gpsimd.memset(spk2b_st[:], 0.0)
    mem2_prev = mem2_st[:, (SGRP - 1) * NO : SGRP * NO]
    spk2b_prev = spk2b_st[:, (SGRP - 1) * NO : SGRP * NO]

    for t in range(T):
        s = t % SGRP
        if s == 0:
            if t > 0:
                cur2_st = l2_pool.tile([128, SGRP * NO], f32, tag="c2")
                mem2_st = l2_pool.tile([128, SGRP * NO], f32, tag="m2")
                spk2_st = l2_pool.tile([128, SGRP * NO], f32, tag="s2")
                spk2b_st = l2_pool.tile([128, SGRP * NO], bf16, tag="s2b")

        # ---- layer 1: cur1 = x_t @ W1.T via 2-term fp16 split ----
        f16 = mybir.dt.float16
        xt = x_pool.tile([128, 2 * 2 * 128], f16, tag="xt")
        nc.sync.dma_start(
            xt[:].rearrange("p (k tb) -> p k tb", k=2),
            x_d[t].rearrange("k p tb -> p k tb"),
        )

        ps1 = ps1_pool.tile([128, NH], f32, tag="ps1")
        for i in range(4):  # (k, term) pairs
            k = i // 2
            nc.tensor.matmul(
                ps1[:],
                lhsT=xt[:, i * 128 : (i + 1) * 128],
                rhs=w1t_sb[:, k * NH : (k + 1) * NH],
                start=(i == 0),
                stop=(i == 3),
            )
        cur1_sb = c1_pool.tile([128, NH], f32, tag="c1")
        nc.scalar.copy(cur1_sb[:], ps1[:])
        nc.sync.dma_start(cur1_d[t], cur1_sb[:])

        # ps1 += -spk1_prev (reset-by-subtract, thresh=1). The accumulation
        # group was closed so cur1 could be read out; has_written bits are
        # still set, so start=False keeps accumulating on HW.
        nc.tensor.matmul(
            ps1[:], lhsT=inegb_sb[:], rhs=spk1b_prev[:],
            start=False, stop=True, skip_group_check=True,
        )

        # mem1 = mem1_prev*beta1 + (cur1 - spk1_prev)   [one fused DVE op]
        mem1 = m1_pool.tile([128, NH], f32, tag="m1")
        nc.vector.scalar_tensor_tensor(
            mem1[:], mem1_prev[:], BETA1, ps1[:], op0=Alu.mult, op1=Alu.add
        )
        nc.gpsimd.dma_start(mem1_d[t], mem1[:])

        # spk1 = mem1 > 1 (bf16 master; f32 cast for DRAM)
        spk1b = s1b_pool.tile([128, NH], bf16, tag="s1b")
        nc.vector.tensor_scalar(spk1b[:], mem1[:], THRESH, None, op0=Alu.is_gt)
        spk1f = s1f_pool.tile([128, NH], f32, tag="s1f")
        nc.vector.tensor_copy(spk1f[:], spk1b[:])
        nc.sync.dma_start(spk1_d[t], spk1f[:])

        # ---- layer 2: cur2 = spk1 @ W2.T ----
        psT = psT_pool.tile([128, 4 * 128], bf16, tag="psT")
        for j in range(4):
            nc.tensor.matmul(
                psT[:, j * 128 : (j + 1) * 128],
                lhsT=spk1b[:, j * 128 : (j + 1) * 128],
                rhs=identb_sb[:],
                is_transpose=True, start=True, stop=True,
            )
        spk1T = s1t_pool.tile([128, 4 * 128], bf16, tag="s1t")
        nc.scalar.copy(spk1T[:], psT[:])

        ps2 = ps2_pool.tile([128, NO], f32, tag="ps2")
        for j in range(4):
            nc.tensor.matmul(
                ps2[:],
                lhsT=spk1T[:, j * 128 : (j + 1) * 128],
                rhs=w2t_sb[:, j * NO : (j + 1) * NO],
                start=(j == 0),
                stop=(j == 3),
            )
        nc.scalar.copy(cur2_st[:, s * NO : (s + 1) * NO], ps2[:])

        nc.tensor.matmul(
            ps2[:], lhsT=inegb_sb[:], rhs=spk2b_prev,
            start=False, stop=True, skip_group_check=True,
        )

        mem2_cur = mem2_st[:, s * NO : (s + 1) * NO]
        nc.vector.scalar_tensor_tensor(
            mem2_cur, mem2_prev, BETA2, ps2[:], op0=Alu.mult, op1=Alu.add
        )
        spk2_cur = spk2_st[:, s * NO : (s + 1) * NO]
        nc.vector.tensor_scalar(spk2_cur, mem2_cur, THRESH, None, op0=Alu.is_gt)
        spk2b_cur = spk2b_st[:, s * NO : (s + 1) * NO]
        nc.vector.tensor_scalar(spk2b_cur, mem2_cur, THRESH, None, op0=Alu.is_gt)

        if s == SGRP - 1:
            t0 = t - SGRP + 1
            for st_tile, dram in ((cur2_st, cur2_d), (mem2_st, mem2_d), (spk2_st, spk2_d)):
                nc.gpsimd.dma_start(
                    dram[t0 : t0 + SGRP].rearrange("t b o -> b t o"),
                    st_tile[:].rearrange("p (t o) -> p t o", t=SGRP),
                )

        mem1_prev, spk1b_prev = mem1, spk1b
        mem2_prev, spk2b_prev = mem2_cur, spk2b_cur


def _shard_inputs(x, W1, W2):
    """Host-side prep: transpose x to [T, NI, B], 2-term fp16 split, shard."""
    bf = ml_dtypes.bfloat16
    xT = np.ascontiguousarray(x.transpose(0, 2, 1))  # [T, NI, B]
    hi = xT.astype(np.float16)
    lo = (xT - hi.astype(np.float32)).astype(np.float16)
    # [T, NI, 2, B] -> [T, 2, 128, 2*B]
    xs = np.stack([hi, lo], axis=2).reshape(T, 2, 128, 2 * B)
    w1t = np.ascontiguousarray(W1.T).astype(np.float16).reshape(2, 128, NH)
    w2t = np.ascontiguousarray(W2.T).astype(bf).reshape(4, 128, NO)
    identb = np.eye(128, dtype=bf)
    inegb = (-np.eye(128)).astype(bf)
    in_maps = []
    for c in range(NCORES):
        # per-core: pick batch columns c*BS..(c+1)*BS from each term block
        xc = xs.reshape(T, 2, 128, 2, B)[:, :, :, :, c * BS : (c + 1) * BS]
        in_maps.append(
            {
                "x": np.ascontiguousarray(xc).reshape(T, 2, 128, 2 * BS),
                "w1t": w1t,
                "w2t": w2t,
                "identb": identb,
                "inegb": inegb,
            }
        )
    return in_maps


def get_program():
    global _CACHE
    if _CACHE is None:
        _CACHE = _build_program()
    return _CACHE


def kernel(x, W1, W2):
    from concourse.bass_utils import run_bass_kernel_spmd

    nc = get_program()
    in_maps = _shard_inputs(np.asarray(x), np.asarray(W1), np.asarray(W2))
    res = run_bass_kernel_spmd(nc, in_maps, core_ids=list(range(NCORES)))
    outs = res.results

    def gather(name):
        return np.concatenate([outs[c][name] for c in range(NCORES)], axis=1)

    return (
        gather("cur1"),
        gather("cur2"),
        gather("spk1"),
        gather("spk2"),
        gather("mem1"),
        gather("mem2"),
    )


# revision 13
# speedup vs baseline: 1.4726x; 1.0344x over previous
"""Trainium2 Bass kernel for a 2-layer LIF spiking net (snntorch-style).

Reference computation (per time step t, for 100 steps):
    cur1 = x_t @ W1.T                  [B, 512]
    mem1 = 0.9*mem1 + cur1 - (mem1_prev > 1)   ; spk1 = mem1 > 1
    cur2 = spk1 @ W2.T                 [B, 10]
    mem2 = 0.8*mem2 + cur2 - (mem2_prev > 1)   ; spk2 = mem2 > 1
    record cur1, cur2, spk1, spk2, mem1, mem2

Sharding: data-parallel over batch (1024 / 8 cores = 128 = partition width).
Weights replicated. Each core runs the full 100-step recurrence on its
batch shard with membrane state resident in SBUF.

Key design points (all verified against the TRN2 cost model):
- x is split on the host into 2 fp16 terms (hi + lo = x to ~2^-22
  relative; fp16 range is ample for N(0,1) data): W1 is exactly {0, 0.5}
  in fp16, so cur1 via 4 fp16 matmuls is fp32-accurate but runs at
  1 cycle/row on the PE instead of fp32's 4.
- reset_t == spk_{t-1} (threshold 1.0, reset-by-subtract), so the previous
  spike tile doubles as the reset mask: one extra -I @ spk_bf16 matmul
  accumulates the reset into PSUM for free.
- The membrane update is ONE fused DVE op: mem = (mem_prev * beta) + psum.
- DMA issue cost is the main scalability limit: loads+cur1 on SyncE HWDGE;
  spk1+mem1 on GpSimd SWDGE (keeps ScalarE's strict FIFO free for the
  PSUM-drain on the recurrence critical path); tiny layer-2 outputs are
  staged in SBUF and flushed once per SGRP steps via SWDGE.
"""

import numpy as np
import ml_dtypes

T, B, NI, NH, NO = 100, 1024, 256, 512, 10
NCORES = 8
BS = B // NCORES  # 128 batch rows per core == SBUF partition count
BETA1, BETA2, THRESH = 0.9, 0.8, 1.0
SGRP = 20  # layer-2 output staging group (steps per DMA)

_CACHE = None


def _build_program(reps=None):
    """Build the Bass program. reps>1 wraps the computation in an on-device
    For_i loop — used only for wall-clock HW timing (amortizes the multi-ms
    PJRT/axon dispatch overhead over many executions)."""
    from contextlib import ExitStack, nullcontext

    import concourse.bass as bass
    import concourse.tile as tile
    from concourse import bacc, mybir

    f32 = mybir.dt.float32
    bf16 = mybir.dt.bfloat16

    nc = bacc.Bacc(
        "TRN2",
        target_bir_lowering=False,
        debug=False,
        enable_asserts=False,
        num_devices=NCORES,
    )

    # ---- DRAM I/O (per-core shard shapes) ----
    # x: [t][k-chunk][p][term,b] with (term,b) merged so each (t,k,p) row is
    # one contiguous 512B run: term-major blocks of 128 fp16 batch values.
    f16 = mybir.dt.float16
    x_d = nc.dram_tensor("x", [T, 2, 128, 2 * BS], f16, kind="ExternalInput").ap()
    w1t_d = nc.dram_tensor("w1t", [2, 128, NH], f16, kind="ExternalInput").ap()
    w2t_d = nc.dram_tensor("w2t", [4, 128, NO], bf16, kind="ExternalInput").ap()
    identb_d = nc.dram_tensor("identb", [128, 128], bf16, kind="ExternalInput").ap()
    inegb_d = nc.dram_tensor("inegb", [128, 128], bf16, kind="ExternalInput").ap()

    cur1_d = nc.dram_tensor("cur1", [T, BS, NH], f32, kind="ExternalOutput").ap()
    spk1_d = nc.dram_tensor("spk1", [T, BS, NH], f32, kind="ExternalOutput").ap()
    mem1_d = nc.dram_tensor("mem1", [T, BS, NH], f32, kind="ExternalOutput").ap()
    cur2_d = nc.dram_tensor("cur2", [T, BS, NO], f32, kind="ExternalOutput").ap()
    spk2_d = nc.dram_tensor("spk2", [T, BS, NO], f32, kind="ExternalOutput").ap()
    mem2_d = nc.dram_tensor("mem2", [T, BS, NO], f32, kind="ExternalOutput").ap()

    with tile.TileContext(nc) as tc:
        with ExitStack() as ctx:
            consts = ctx.enter_context(tc.tile_pool(name="consts", bufs=1))
            x_pool = ctx.enter_context(tc.tile_pool(name="x", bufs=6))
            m1_pool = ctx.enter_context(tc.tile_pool(name="m1", bufs=4))
            s1b_pool = ctx.enter_context(tc.tile_pool(name="s1b", bufs=4))
            s1f_pool = ctx.enter_context(tc.tile_pool(name="s1f", bufs=4))
            c1_pool = ctx.enter_context(tc.tile_pool(name="c1", bufs=4))
            s1t_pool = ctx.enter_context(tc.tile_pool(name="s1t", bufs=3))
            l2_pool = ctx.enter_context(tc.tile_pool(name="l2", bufs=3))
            ps1_pool = ctx.enter_context(tc.tile_pool(name="ps1", bufs=2, space="PSUM"))
            psT_pool = ctx.enter_context(tc.tile_pool(name="psT", bufs=2, space="PSUM"))
            ps2_pool = ctx.enter_context(tc.tile_pool(name="ps2", bufs=2, space="PSUM"))

            # ---- constants ----
            f16 = mybir.dt.float16
            w1t_sb = consts.tile([128, 2 * NH], f16)
            for k in range(2):
                nc.sync.dma_start(w1t_sb[:, k * NH : (k + 1) * NH], w1t_d[k])
            w2t_sb = consts.tile([128, 4 * NO], bf16)
            for j in range(4):
                nc.sync.dma_start(w2t_sb[:, j * NO : (j + 1) * NO], w2t_d[j])
            identb_sb = consts.tile([128, 128], bf16)
            nc.sync.dma_start(identb_sb[:], identb_d[:])
            inegb_sb = consts.tile([128, 128], bf16)
            nc.sync.dma_start(inegb_sb[:], inegb_d[:])

            loop_cm = tc.For_i(0, reps, 1) if reps and reps > 1 else nullcontext()
            with loop_cm:
                _emit_body(
                    nc, tc, mybir,
                    (x_pool, m1_pool, s1b_pool, s1f_pool, c1_pool, s1t_pool,
                     l2_pool, ps1_pool, psT_pool, ps2_pool),
                    (w1t_sb, w2t_sb, identb_sb, inegb_sb),
                    (x_d, cur1_d, spk1_d, mem1_d, cur2_d, spk2_d, mem2_d),
                )

    nc.compile()
    return nc


def _emit_body(nc, tc, mybir, pools, consts_sb, drams):
    f32 = mybir.dt.float32
    bf16 = mybir.dt.bfloat16
    Alu = mybir.AluOpType
    (x_pool, m1_pool, s1b_pool, s1f_pool, c1_pool, s1t_pool,
     l2_pool, ps1_pool, psT_pool, ps2_pool) = pools
    w1t_sb, w2t_sb, identb_sb, inegb_sb = consts_sb
    x_d, cur1_d, spk1_d, mem1_d, cur2_d, spk2_d, mem2_d = drams

    # ---- initial state (zeros) ----
    mem1_prev = m1_pool.tile([128, NH], f32, tag="m1")
    nc.gpsimd.memset(mem1_prev[:], 0.0)
    spk1b_prev = s1b_pool.tile([128, NH], bf16, tag="s1b")
    nc.gpsimd.memset(spk1b_prev[:], 0.0)
    # layer-2 staging tiles: SGRP steps wide, written in-place per step,
    # flushed to DRAM once per group via SWDGE
    cur2_st = l2_pool.tile([128, SGRP * NO], f32, tag="c2")
    mem2_st = l2_pool.tile([128, SGRP * NO], f32, tag="m2")
    spk2_st = l2_pool.tile([128, SGRP * NO], f32, tag="s2")
    spk2b_st = l2_pool.tile([128, SGRP * NO], bf16, tag="s2b")
    nc.gpsimd.memset(mem2_st[:], 0.0)
    nc.gpsimd.memset(spk2b_st[:], 0.0)
    mem2_prev = mem2_st[:, (SGRP - 1) * NO : SGRP * NO]
    spk2b_prev = spk2b_st[:, (SGRP - 1) * NO : SGRP * NO]

    for t in range(T):
        s = t % SGRP
        if s == 0:
            if t > 0:
                cur2_st = l2_pool.tile([128, SGRP * NO], f32, tag="c2")
                mem2_st = l2_pool.tile([128, SGRP * NO], f32, tag="m2")
                spk2_st = l2_pool.tile([128, SGRP * NO], f32, tag="s2")
                spk2b_st = l2_pool.tile([128, SGRP * NO], bf16, tag="s2b")

        # ---- layer 1: cur1 = x_t @ W1.T via 2-term fp16 split ----
        f16 = mybir.dt.float16
        xt = x_pool.tile([128, 2 * 2 * 128], f16, tag="xt")
        nc.sync.dma_start(
            xt[:].rearrange("p (k tb) -> p k tb", k=2),
            x_d[t].rearrange("k p tb -> p k tb"),
        )

        ps1 = ps1_pool.tile([128, NH], f32, tag="ps1")
        for i in range(4):  # (k, term) pairs
            k = i // 2
            nc.tensor.matmul(
                ps1[:],
                lhsT=xt[:, i * 128 : (i + 1) * 128],
                rhs=w1t_sb[:, k * NH : (k + 1) * NH],
                start=(i == 0),
                stop=(i == 3),
            )
        cur1_sb = c1_pool.tile([128, NH], f32, tag="c1")
        nc.scalar.copy(cur1_sb[:], ps1[:])
        nc.sync.dma_start(cur1_d[t], cur1_sb[:])

        # ps1 += -spk1_prev (reset-by-subtract, thresh=1). The accumulation
        # group was closed so cur1 could be read out; has_written bits are
        # still set, so start=False keeps accumulating on HW.
        nc.tensor.matmul(
            ps1[:], lhsT=inegb_sb[:], rhs=spk1b_prev[:],
            start=False, stop=True, skip_group_check=True,
        )

        # mem1 = mem1_prev*beta1 + (cur1 - spk1_prev)   [one fused DVE op]
        mem1 = m1_pool.tile([128, NH], f32, tag="m1")
        nc.vector.scalar_tensor_tensor(
            mem1[:], mem1_prev[:], BETA1, ps1[:], op0=Alu.mult, op1=Alu.add
        )
        nc.sync.dma_start(mem1_d[t], mem1[:])

        # spk1 = mem1 > 1 (bf16 master; f32 cast for DRAM)
        spk1b = s1b_pool.tile([128, NH], bf16, tag="s1b")
        nc.vector.tensor_scalar(spk1b[:], mem1[:], THRESH, None, op0=Alu.is_gt)
        spk1f = s1f_pool.tile([128, NH], f32, tag="s1f")
        nc.vector.tensor_copy(spk1f[:], spk1b[:])
        nc.sync.dma_start(spk1_d[t], spk1f[:])

        # ---- layer 2: cur2 = spk1 @ W2.T ----
        psT = psT_pool.tile([128, 4 * 128], bf16, tag="psT")
        for j in range(4):
            nc.tensor.matmul(
                psT[:, j * 128 : (j + 1) * 128],
                lhsT=spk1b[:, j * 128 : (j + 1) * 128],
                rhs=identb_sb[:],
                is_transpose=True, start=True, stop=True,
            )
        spk1T = s1t_pool.tile([128, 4 * 128], bf16, tag="s1t")
        nc.scalar.copy(spk1T[:], psT[:])

        ps2 = ps2_pool.tile([128, NO], f32, tag="ps2")
        for j in range(4):
            nc.tensor.matmul(
                ps2[:],
                lhsT=spk1T[:, j * 128 : (j + 1) * 128],
                rhs=w2t_sb[:, j * NO : (j + 1) * NO],
                start=(j == 0),
                stop=(j == 3),
            )
        nc.scalar.copy(cur2_st[:, s * NO : (s + 1) * NO], ps2[:])

        nc.tensor.matmul(
            ps2[:], lhsT=inegb_sb[:], rhs=spk2b_prev,
            start=False, stop=True, skip_group_check=True,
        )

        mem2_cur = mem2_st[:, s * NO : (s + 1) * NO]
        nc.vector.scalar_tensor_tensor(
            mem2_cur, mem2_prev, BETA2, ps2[:], op0=Alu.mult, op1=Alu.add
        )
        spk2_cur = spk2_st[:, s * NO : (s + 1) * NO]
        nc.vector.tensor_scalar(spk2_cur, mem2_cur, THRESH, None, op0=Alu.is_gt)
        spk2b_cur = spk2b_st[:, s * NO : (s + 1) * NO]
        nc.vector.tensor_scalar(spk2b_cur, mem2_cur, THRESH, None, op0=Alu.is_gt)

        if s == SGRP - 1:
            t0 = t - SGRP + 1
            for st_tile, dram in ((cur2_st, cur2_d), (mem2_st, mem2_d), (spk2_st, spk2_d)):
                nc.gpsimd.dma_start(
                    dram[t0 : t0 + SGRP].rearrange("t b o -> b t o"),
                    st_tile[:].rearrange("p (t o) -> p t o", t=SGRP),
                )

        mem1_prev, spk1b_prev = mem1, spk1b
        mem2_prev, spk2b_prev = mem2_cur, spk2b_cur


def _shard_inputs(x, W1, W2):
    """Host-side prep: transpose x to [T, NI, B], 2-term fp16 split, shard."""
    bf = ml_dtypes.bfloat16
    xT = np.ascontiguousarray(x.transpose(0, 2, 1))  # [T, NI, B]
    hi = xT.astype(np.float16)
    lo = (xT - hi.astype(np.float32)).astype(np.float16)
    # [T, NI, 2, B] -> [T, 2, 128, 2*B]
    xs = np.stack([hi, lo], axis=2).reshape(T, 2, 128, 2 * B)
    w1t = np.ascontiguousarray(W1.T).astype(np.float16).reshape(2, 128, NH)
    w2t = np.ascontiguousarray(W2.T).astype(bf).reshape(4, 128, NO)
    identb = np.eye(128, dtype=bf)
    inegb = (-np.eye(128)).astype(bf)
    in_maps = []
    for c in range(NCORES):
        # per-core: pick batch columns c*BS..(c+1)*BS from each term block
        xc = xs.reshape(T, 2, 128, 2, B)[:, :, :, :, c * BS : (c + 1) * BS]
        in_maps.append(
            {
                "x": np.ascontiguousarray(xc).reshape(T, 2, 128, 2 * BS),
                "w1t": w1t,
                "w2t": w2t,
                "identb": identb,
                "inegb": inegb,
            }
        )
    return in_maps


def get_program():
    global _CACHE
    if _CACHE is None:
        _CACHE = _build_program()
    return _CACHE


def kernel(x, W1, W2):
    from concourse.bass_utils import run_bass_kernel_spmd

    nc = get_program()
    in_maps = _shard_inputs(np.asarray(x), np.asarray(W1), np.asarray(W2))
    res = run_bass_kernel_spmd(nc, in_maps, core_ids=list(range(NCORES)))
    outs = res.results

    def gather(name):
        return np.concatenate([outs[c][name] for c in range(NCORES)], axis=1)

    return (
        gather("cur1"),
        gather("cur2"),
        gather("spk1"),
        gather("spk2"),
        gather("mem1"),
        gather("mem2"),
    )
